# revision 13
# baseline (speedup 1.0000x reference)
"""Trainium2 Bass kernel for per-sample channel-modulated 3x3 conv (CoModConv).

Math (matches the reference nn.Module):
    s = lrelu(lrelu(lrelu(y @ w0.T + b0) @ w1.T + b1) @ w2.T + b2)   # (B, C_in)
    out = conv3x3(x * s[:, :, None, None], conv_w, pad=1)            # (B, C_out, H, W)

Strategy: data-parallel over batch, 2 samples per NeuronCore (8 cores).
Per core:
  - x (bf16, zero-padded to the 66x66 grid on host) lands in SBUF with one
    contiguous DMA per (sample, ci-tile).
  - All style-MLP params arrive in a single packed DMA; the MLP runs in fp32
    on the tensor engine; each layer's bias + leaky-relu is one scalar-engine
    Prelu activation.
  - Modulation is folded into the conv weights: w_mod = conv_w * s[b, ci]
    via per-partition-scaled fp32->bf16 activations, split by co-half so the
    first conv chain starts as soon as its half of the weights has landed.
  - The conv is 18 accumulating 128x128x512 bf16 matmuls per PSUM tile,
    one tile per 8 output rows, using 2D (row, col) rhs access patterns so
    only valid output columns are computed.
  - PSUM drains to a compact fp32 output tile; output DMAs are chunked per
    8-row group so the store pipeline finishes right behind the last matmul.
Host-side work is layout-only (transpose / reshape / pad / dtype cast).
"""

import numpy as np
import ml_dtypes

B, D_CAT, C_IN, C_OUT, K, H, W = 16, 512, 256, 256, 3, 64, 64
NCORES = 8
BL = B // NCORES          # samples per core (2)
CIT = C_IN // 128         # ci tiles (2)
COT = C_OUT // 128        # co tiles (2)
GW = W + 2                # padded grid width (66)
GH = H + 2                # padded grid height (66)
RG = 8                    # output rows per PSUM tile (8*64 = 512 columns)
NT = H // RG              # row-group tiles per (sample, co-tile) (8)
KKT = K * K * CIT         # matmuls per accumulation chain (18)
WHALF = K * K * 128       # weight columns per (ci_t, co_t) half (1152)

# packed MLP-param column offsets (per partition, fp32).
# pp1 = layer-1 deps (y, w0, b0); pp2 = later layers (w1, w2, b1, b2)
_PY = 0                       # y^T:   4 k-tiles x BL
_PW0 = _PY + 4 * BL           # w0^T:  4 k-tiles x 256
_PB0 = _PW0 + 4 * C_IN        # b0: CIT
_P1TOT = _PB0 + CIT
_PW1 = 0                      # w1^T:  2 k-tiles x 256
_PW2 = _PW1 + 2 * C_IN        # w2^T:  2 k-tiles x 256
_PB12 = _PW2 + 2 * C_IN       # b1,b2: 2 x CIT
_P2TOT = _PB12 + 2 * CIT

_BF16 = ml_dtypes.bfloat16
_COMPILED = None


def _build():
    import concourse.mybir as mybir
    import concourse.tile as tile
    from concourse import bacc

    bf16 = mybir.dt.bfloat16
    f32 = mybir.dt.float32
    Prelu = mybir.ActivationFunctionType.Prelu

    nc = bacc.Bacc("TRN2", target_bir_lowering=False, debug=False, num_devices=NCORES)

    pp1_in = nc.declare_dram_parameter("pp1", [128, _P1TOT], f32, isOutput=False)
    pp2_in = nc.declare_dram_parameter("pp2", [128, _P2TOT], f32, isOutput=False)
    wf_in = nc.declare_dram_parameter("wf", [CIT, COT, 128, WHALF], f32, isOutput=False)
    xb_in = nc.declare_dram_parameter("xb", [BL, CIT, 128, GH * GW], bf16, isOutput=False)
    out_ext = nc.declare_dram_parameter("out", [BL, COT, 128, H * W], f32, isOutput=True)

    with tile.TileContext(nc) as tc:
        with (
            tc.tile_pool(name="const", bufs=1) as cpool,
            tc.tile_pool(name="xpad", bufs=4) as padpool,
            tc.tile_pool(name="wmod", bufs=8) as wmpool,
            tc.tile_pool(name="osb", bufs=3) as opool,
            tc.tile_pool(name="mpsum", bufs=2, space="PSUM") as mpsum,
            tc.tile_pool(name="cpsum", bufs=6, space="PSUM") as cpsum,
        ):
            # ---- DMAs, ordered by first use ----
            pp1_sb = cpool.tile([128, _P1TOT], f32)
            nc.sync.dma_start(pp1_sb[:], pp1_in[:])
            pp2_sb = cpool.tile([128, _P2TOT], f32)
            nc.sync.dma_start(pp2_sb[:], pp2_in[:])

            wf_sbs = {}
            for co_t in range(COT):          # co0 halves first: first chains use co_t=0
                for ci_t in range(CIT):
                    t = cpool.tile([128, WHALF], f32, tag=f"wf{ci_t}{co_t}")
                    nc.sync.dma_start(t[:], wf_in[ci_t, co_t])
                    wf_sbs[(ci_t, co_t)] = t
                if co_t == 0:
                    grids = {}
                    for b in range(BL):
                        for ci_t in range(CIT):
                            t = padpool.tile([128, GH * GW], bf16)
                            nc.sync.dma_start(t[:], xb_in[b, ci_t])
                            grids[(b, ci_t)] = t[:].rearrange("p (a b) -> p a b", b=GW)

            # ---- style MLP (fp32): s^T per ci-tile in SBUF ----
            def mlp_layer(rhs_of_kt, kts, w_sb, w_base, bias_ap, out_sb):
                for ct in range(CIT):
                    ps = mpsum.tile([128, BL], f32, tag="mps")
                    for kt in range(kts):
                        nc.tensor.matmul(
                            ps[:],
                            w_sb[:, w_base + kt * C_IN + ct * 128 :][:, :128],
                            rhs_of_kt(kt),
                            start=(kt == 0),
                            stop=(kt == kts - 1),
                        )
                    nc.scalar.activation(
                        out_sb[:, ct * BL : (ct + 1) * BL],
                        ps[:],
                        Prelu,
                        bias=bias_ap(ct),
                        scale=1.0,
                        alpha=0.01,
                    )

            s0_sb = cpool.tile([128, CIT * BL], f32)
            s1_sb = cpool.tile([128, CIT * BL], f32)
            s_sb = cpool.tile([128, CIT * BL], f32)
            mlp_layer(
                lambda kt: pp1_sb[:, _PY + kt * BL : _PY + (kt + 1) * BL],
                4, pp1_sb, _PW0,
                lambda ct: pp1_sb[:, _PB0 + ct : _PB0 + ct + 1],
                s0_sb,
            )
            mlp_layer(
                lambda kt: s0_sb[:, kt * BL : (kt + 1) * BL],
                2, pp2_sb, _PW1,
                lambda ct: pp2_sb[:, _PB12 + ct : _PB12 + ct + 1],
                s1_sb,
            )
            mlp_layer(
                lambda kt: s1_sb[:, kt * BL : (kt + 1) * BL],
                2, pp2_sb, _PW2,
                lambda ct: pp2_sb[:, _PB12 + CIT + ct : _PB12 + CIT + ct + 1],
                s_sb,
            )

            # ---- modulated weights: w_mod[b, ci_t, co_t] = wf * s[b, ci] (bf16) ----
            w_mods = {}
            for b in range(BL):
                for co_t in range(COT):
                    for ci_t in range(CIT):
                        t = wmpool.tile([128, WHALF], bf16)
                        nc.scalar.mul(
                            t[:],
                            wf_sbs[(ci_t, co_t)][:],
                            s_sb[:, ci_t * BL + b : ci_t * BL + b + 1],
                        )
                        w_mods[(b, ci_t, co_t)] = t

            # ---- conv: per (sample, co-tile, 8-row group): 18 accumulating matmuls ----
            for b in range(BL):
                for co_t in range(COT):
                    o_sb = opool.tile([128, H * W], f32, tag="osb")
                    for n in range(NT):
                        ps = cpsum.tile([128, RG * W], f32, tag="cps")
                        i = 0
                        for ci_t in range(CIT):
                            g = grids[(b, ci_t)]
                            wm = w_mods[(b, ci_t, co_t)]
                            for ki in range(K):
                                for kj in range(K):
                                    nc.tensor.matmul(
                                        ps[:],
                                        wm[:, (ki * K + kj) * 128 : (ki * K + kj + 1) * 128],
                                        g[:, RG * n + ki : RG * n + ki + RG, kj : kj + W],
                                        start=(i == 0),
                                        stop=(i == KKT - 1),
                                    )
                                    i += 1
                        nc.vector.tensor_copy(o_sb[:, RG * W * n : RG * W * (n + 1)], ps[:])
                        nc.sync.dma_start(
                            out_ext[b, co_t][:, RG * W * n : RG * W * (n + 1)],
                            o_sb[:, RG * W * n : RG * W * (n + 1)],
                        )

    nc.compile()
    return nc


def _get_nc():
    global _COMPILED
    if _COMPILED is None:
        _COMPILED = _build()
    return _COMPILED


def _prep_in_maps(x, y, w0, b0, w1, b1, w2, b2, conv_w):
    x = np.ascontiguousarray(x, dtype=np.float32)
    y = np.ascontiguousarray(y, dtype=np.float32)

    # packed per-core-invariant params: pp1 = (y, w0, b0), pp2 = (w1, w2, b1, b2)
    pp1_shared = np.empty((128, _P1TOT), dtype=np.float32)
    pp1_shared[:, _PW0 : _PW0 + 4 * C_IN] = (
        w0.astype(np.float32).T.reshape(4, 128, C_IN).transpose(1, 0, 2).reshape(128, 4 * C_IN)
    )
    pp1_shared[:, _PB0 : _PB0 + CIT] = b0.astype(np.float32).reshape(CIT, 128).T
    pp2 = np.empty((128, _P2TOT), dtype=np.float32)
    pp2[:, _PW1 : _PW1 + 2 * C_IN] = (
        w1.astype(np.float32).T.reshape(2, 128, C_IN).transpose(1, 0, 2).reshape(128, 2 * C_IN)
    )
    pp2[:, _PW2 : _PW2 + 2 * C_IN] = (
        w2.astype(np.float32).T.reshape(2, 128, C_IN).transpose(1, 0, 2).reshape(128, 2 * C_IN)
    )
    for i, bb in enumerate((b1, b2)):
        pp2[:, _PB12 + i * CIT : _PB12 + (i + 1) * CIT] = (
            bb.astype(np.float32).reshape(CIT, 128).T
        )

    # conv weights: (co_t, co, ci_t, ci, ki, kj) -> (ci_t, co_t, ci, (ki kj) co)
    wf = np.ascontiguousarray(
        conv_w.astype(np.float32)
        .reshape(COT, 128, CIT, 128, K, K)
        .transpose(2, 0, 3, 4, 5, 1)
        .reshape(CIT, COT, 128, WHALF)
    )

    xb_all = np.zeros((B, CIT, 128, GH, GW), dtype=_BF16)
    xb_all[:, :, :, 1 : H + 1, 1 : W + 1] = x.reshape(B, CIT, 128, H, W)
    xb_all = xb_all.reshape(B, CIT, 128, GH * GW)

    in_maps = []
    for c in range(NCORES):
        sl = slice(c * BL, (c + 1) * BL)
        pp1 = pp1_shared.copy()
        pp1[:, _PY : _PY + 4 * BL] = (
            y[sl].T.reshape(4, 128, BL).transpose(1, 0, 2).reshape(128, 4 * BL)
        )
        in_maps.append(
            {
                "pp1": pp1,
                "pp2": pp2,
                "wf": wf,
                "xb": np.ascontiguousarray(xb_all[sl]),
            }
        )
    return in_maps


def _run(in_maps, trace=False):
    from concourse.bass_utils import run_bass_kernel_spmd

    nc = _get_nc()
    res = run_bass_kernel_spmd(nc, in_maps, list(range(NCORES)), trace=trace)
    out = np.concatenate(
        [res.results[c]["out"].reshape(BL, C_OUT, H, W) for c in range(NCORES)], axis=0
    ).astype(np.float32, copy=False)
    return out, res


def kernel(x, y, w0, b0, w1, b1, w2, b2, conv_w):
    in_maps = _prep_in_maps(x, y, w0, b0, w1, b1, w2, b2, conv_w)
    out, _ = _run(in_maps, trace=False)
    return out


# revision 34
# speedup vs baseline: 1.0135x; 1.0135x over previous
"""Trainium2 Bass kernel for per-sample channel-modulated 3x3 conv (CoModConv).

Math (matches the reference nn.Module):
    s = lrelu(lrelu(lrelu(y @ w0.T + b0) @ w1.T + b1) @ w2.T + b2)   # (B, C_in)
    out = conv3x3(x * s[:, :, None, None], conv_w, pad=1)            # (B, C_out, H, W)

Strategy: data-parallel over batch, 2 samples per NeuronCore (8 cores).
Per core:
  - x (bf16, zero-padded to the 66x66 grid on host) lands in SBUF with one
    contiguous DMA per (sample, ci-tile).
  - All style-MLP params arrive in a single packed DMA; the MLP runs in fp32
    on the tensor engine; each layer's bias + leaky-relu is one scalar-engine
    Prelu activation.
  - Modulation is folded into the conv weights: w_mod = conv_w * s[b, ci]
    via per-partition-scaled fp32->bf16 activations, split by co-half so the
    first conv chain starts as soon as its half of the weights has landed.
  - The conv is 18 accumulating 128x128x512 bf16 matmuls per PSUM tile,
    one tile per 8 output rows, using 2D (row, col) rhs access patterns so
    only valid output columns are computed.
  - PSUM drains to a compact fp32 output tile; output DMAs are chunked per
    8-row group so the store pipeline finishes right behind the last matmul.
Host-side work is layout-only (transpose / reshape / pad / dtype cast).
"""

import numpy as np
import ml_dtypes

B, D_CAT, C_IN, C_OUT, K, H, W = 16, 512, 256, 256, 3, 64, 64
NCORES = 8
BL = B // NCORES          # samples per core (2)
CIT = C_IN // 128         # ci tiles (2)
COT = C_OUT // 128        # co tiles (2)
GW = W + 2                # padded grid width (66)
GH = H + 2                # padded grid height (66)
RG = 8                    # output rows per PSUM tile (8*64 = 512 columns)
NT = H // RG              # row-group tiles per (sample, co-tile) (8)
KKT = K * K * CIT         # matmuls per accumulation chain (18)
WHALF = K * K * 128       # weight columns per (ci_t, co_t) half (1152)

# packed MLP-param column offsets (per partition, fp32).
# pp1 = layer-1 deps (y, w0, b0); pp2 = later layers (w1, w2, b1, b2)
_PY = 0                       # y^T:   4 k-tiles x BL
_PW0 = _PY + 4 * BL           # w0^T:  4 k-tiles x 256
_PB0 = _PW0 + 4 * C_IN        # b0: CIT
_P1TOT = _PB0 + CIT
_PW1 = 0                      # w1^T:  2 k-tiles x 256
_PW2 = _PW1 + 2 * C_IN        # w2^T:  2 k-tiles x 256
_PB12 = _PW2 + 2 * C_IN       # b1,b2: 2 x CIT
_P2TOT = _PB12 + 2 * CIT

_BF16 = ml_dtypes.bfloat16
_COMPILED = None


def _build():
    import concourse.mybir as mybir
    import concourse.tile as tile
    from concourse import bacc

    bf16 = mybir.dt.bfloat16
    f32 = mybir.dt.float32
    Prelu = mybir.ActivationFunctionType.Prelu

    nc = bacc.Bacc("TRN2", target_bir_lowering=False, debug=False, num_devices=NCORES)

    pp1_in = nc.declare_dram_parameter("pp1", [128, _P1TOT], f32, isOutput=False)
    pp2_in = nc.declare_dram_parameter("pp2", [128, _P2TOT], f32, isOutput=False)
    wf_in = nc.declare_dram_parameter("wf", [CIT, COT, 128, WHALF], bf16, isOutput=False)
    xb_in = nc.declare_dram_parameter("xb", [BL, CIT, 128, GH * GW], bf16, isOutput=False)
    out_ext = nc.declare_dram_parameter("out", [BL, COT, 128, H * W], f32, isOutput=True)

    with tile.TileContext(nc) as tc:
        with (
            tc.tile_pool(name="const", bufs=1) as cpool,
            tc.tile_pool(name="xpad", bufs=4) as padpool,
            tc.tile_pool(name="wmod", bufs=8) as wmpool,
            tc.tile_pool(name="osb", bufs=3) as opool,
            tc.tile_pool(name="cpsum", bufs=8, space="PSUM") as cpsum,
        ):
            # warm the scalar-engine activation table before the params land
            # so the first real Prelu doesn't pay the LoadActFuncSet latency
            warm = cpool.tile([128, 1], f32)
            nc.vector.memset(warm[:], 0.0)
            nc.scalar.activation(warm[:], warm[:], Prelu, bias=warm[:], scale=1.0, alpha=0.01)

            # ---- DMAs, ordered by first use ----
            pp1_sb = cpool.tile([128, _P1TOT], f32)
            nc.sync.dma_start(pp1_sb[:], pp1_in[:])
            pp2_sb = cpool.tile([128, _P2TOT], f32)
            nc.sync.dma_start(pp2_sb[:], pp2_in[:])

            wf_sbs = {}
            for co_t in range(COT):          # co0 halves first: first chains use co_t=0
                for ci_t in range(CIT):
                    t = cpool.tile([128, WHALF], bf16, tag=f"wf{ci_t}{co_t}")
                    nc.sync.dma_start(t[:], wf_in[ci_t, co_t])
                    wf_sbs[(ci_t, co_t)] = t
                if co_t == 0:
                    grids = {}
                    for b in range(BL):
                        for ci_t in range(CIT):
                            t = padpool.tile([128, GH * GW], bf16)
                            nc.sync.dma_start(t[:], xb_in[b, ci_t])
                            grids[(b, ci_t)] = t[:].rearrange("p (a b) -> p a b", b=GW)

            # ---- style MLP (fp32): s^T per ci-tile in SBUF ----
            def mlp_layer(rhs_of_kt, kts, w_sb, w_base, bias_ap, out_sb):
                for ct in range(CIT):
                    mps = cpsum.tile([128, RG * W], f32, tag="cps")
                    for kt in range(kts):
                        nc.tensor.matmul(
                            mps[:, :BL],
                            w_sb[:, w_base + kt * C_IN + ct * 128 :][:, :128],
                            rhs_of_kt(kt),
                            start=(kt == 0),
                            stop=(kt == kts - 1),
                        )
                    nc.scalar.activation(
                        out_sb[:, ct * BL : (ct + 1) * BL],
                        mps[:, :BL],
                        Prelu,
                        bias=bias_ap(ct),
                        scale=1.0,
                        alpha=0.01,
                    )

            s0_sb = cpool.tile([128, CIT * BL], f32)
            s1_sb = cpool.tile([128, CIT * BL], f32)
            s_sb = cpool.tile([128, CIT * BL], f32)
            mlp_layer(
                lambda kt: pp1_sb[:, _PY + kt * BL : _PY + (kt + 1) * BL],
                4, pp1_sb, _PW0,
                lambda ct: pp1_sb[:, _PB0 + ct : _PB0 + ct + 1],
                s0_sb,
            )
            mlp_layer(
                lambda kt: s0_sb[:, kt * BL : (kt + 1) * BL],
                2, pp2_sb, _PW1,
                lambda ct: pp2_sb[:, _PB12 + ct : _PB12 + ct + 1],
                s1_sb,
            )
            mlp_layer(
                lambda kt: s1_sb[:, kt * BL : (kt + 1) * BL],
                2, pp2_sb, _PW2,
                lambda ct: pp2_sb[:, _PB12 + CIT + ct : _PB12 + CIT + ct + 1],
                s_sb,
            )

            # ---- modulated weights: w_mod[b, ci_t, co_t] = wf * s[b, ci] (bf16) ----
            w_mods = {}
            for b in range(BL):
                for co_t in range(COT):
                    for ci_t in range(CIT):
                        t = wmpool.tile([128, WHALF], bf16)
                        nc.vector.tensor_scalar_mul(
                            t[:],
                            wf_sbs[(ci_t, co_t)][:],
                            s_sb[:, ci_t * BL + b : ci_t * BL + b + 1],
                        )
                        w_mods[(b, ci_t, co_t)] = t

            # ---- conv: per (sample, co-tile): weight-outer over all 8 PSUM banks,
            # so each stationary weight tile is loaded once per 8 matmuls ----
            def conv_group(b, co_t, o_sb, ns):
                # weight-outer over the row-groups in `ns`: each stationary
                # weight tile is loaded once per len(ns) matmuls
                pss = [
                    cpsum.tile([128, RG * W], f32, name=f"cps_{b}_{co_t}_{n}", tag="cps")
                    for n in ns
                ]
                q = 0
                for ci_t in range(CIT):
                    g = grids[(b, ci_t)]
                    wm = w_mods[(b, ci_t, co_t)]
                    for ki in range(K):
                        for kj in range(K):
                            wt = wm[:, (ki * K + kj) * 128 : (ki * K + kj + 1) * 128]
                            for j, n in enumerate(ns):
                                nc.tensor.matmul(
                                    pss[j][:],
                                    wt,
                                    g[:, RG * n + ki : RG * n + ki + RG, kj : kj + W],
                                    start=(q == 0),
                                    stop=(q == KKT - 1),
                                )
                            q += 1
                for j, n in enumerate(ns):
                    copy = nc.vector.tensor_copy if n % 2 == 0 else nc.scalar.copy
                    copy(o_sb[:, RG * W * n : RG * W * (n + 1)], pss[j][:])
                    nc.sync.dma_start(
                        out_ext[b, co_t][:, RG * W * n : RG * W * (n + 1)],
                        o_sb[:, RG * W * n : RG * W * (n + 1)],
                    )

            for b in range(BL):
                for co_t in range(COT):
                    o_sb = opool.tile([128, H * W], f32, tag="osb")
                    last = b == BL - 1 and co_t == COT - 1
                    if last:
                        # split the final group so its drain/store burst
                        # overlaps the second half's matmuls
                        conv_group(b, co_t, o_sb, list(range(NT - 2)))
                        conv_group(b, co_t, o_sb, [NT - 2, NT - 1])
                    else:
                        conv_group(b, co_t, o_sb, list(range(NT)))

    nc.compile()
    return nc


def _get_nc():
    global _COMPILED
    if _COMPILED is None:
        _COMPILED = _build()
    return _COMPILED


def _prep_in_maps(x, y, w0, b0, w1, b1, w2, b2, conv_w):
    x = np.ascontiguousarray(x, dtype=np.float32)
    y = np.ascontiguousarray(y, dtype=np.float32)

    # packed per-core-invariant params: pp1 = (y, w0, b0), pp2 = (w1, w2, b1, b2)
    pp1_shared = np.empty((128, _P1TOT), dtype=np.float32)
    pp1_shared[:, _PW0 : _PW0 + 4 * C_IN] = (
        w0.astype(np.float32).T.reshape(4, 128, C_IN).transpose(1, 0, 2).reshape(128, 4 * C_IN)
    )
    pp1_shared[:, _PB0 : _PB0 + CIT] = b0.astype(np.float32).reshape(CIT, 128).T
    pp2 = np.empty((128, _P2TOT), dtype=np.float32)
    pp2[:, _PW1 : _PW1 + 2 * C_IN] = (
        w1.astype(np.float32).T.reshape(2, 128, C_IN).transpose(1, 0, 2).reshape(128, 2 * C_IN)
    )
    pp2[:, _PW2 : _PW2 + 2 * C_IN] = (
        w2.astype(np.float32).T.reshape(2, 128, C_IN).transpose(1, 0, 2).reshape(128, 2 * C_IN)
    )
    for i, bb in enumerate((b1, b2)):
        pp2[:, _PB12 + i * CIT : _PB12 + (i + 1) * CIT] = (
            bb.astype(np.float32).reshape(CIT, 128).T
        )

    # conv weights: (co_t, co, ci_t, ci, ki, kj) -> (ci_t, co_t, ci, (ki kj) co)
    wf = np.ascontiguousarray(
        conv_w.astype(np.float32)
        .reshape(COT, 128, CIT, 128, K, K)
        .transpose(2, 0, 3, 4, 5, 1)
        .reshape(CIT, COT, 128, WHALF)
    ).astype(_BF16)

    xb_all = np.zeros((B, CIT, 128, GH, GW), dtype=_BF16)
    xb_all[:, :, :, 1 : H + 1, 1 : W + 1] = x.reshape(B, CIT, 128, H, W)
    xb_all = xb_all.reshape(B, CIT, 128, GH * GW)

    in_maps = []
    for c in range(NCORES):
        sl = slice(c * BL, (c + 1) * BL)
        pp1 = pp1_shared.copy()
        pp1[:, _PY : _PY + 4 * BL] = (
            y[sl].T.reshape(4, 128, BL).transpose(1, 0, 2).reshape(128, 4 * BL)
        )
        in_maps.append(
            {
                "pp1": pp1,
                "pp2": pp2,
                "wf": wf,
                "xb": np.ascontiguousarray(xb_all[sl]),
            }
        )
    return in_maps


def _run(in_maps, trace=False):
    from concourse.bass_utils import run_bass_kernel_spmd

    nc = _get_nc()
    res = run_bass_kernel_spmd(nc, in_maps, list(range(NCORES)), trace=trace)
    out = np.concatenate(
        [res.results[c]["out"].reshape(BL, C_OUT, H, W) for c in range(NCORES)], axis=0
    ).astype(np.float32, copy=False)
    return out, res


def kernel(x, y, w0, b0, w1, b1, w2, b2, conv_w):
    in_maps = _prep_in_maps(x, y, w0, b0, w1, b1, w2, b2, conv_w)
    out, _ = _run(in_maps, trace=False)
    return out


# revision 41
# speedup vs baseline: 1.0160x; 1.0024x over previous
"""Trainium2 Bass kernel for per-sample channel-modulated 3x3 conv (CoModConv).

Math (matches the reference nn.Module):
    s = lrelu(lrelu(lrelu(y @ w0.T + b0) @ w1.T + b1) @ w2.T + b2)   # (B, C_in)
    out = conv3x3(x * s[:, :, None, None], conv_w, pad=1)            # (B, C_out, H, W)

Strategy: data-parallel over batch, 2 samples per NeuronCore (8 cores).
Per core:
  - x (bf16, zero-padded to the 66x66 grid on host) lands in SBUF with one
    contiguous DMA per (sample, ci-tile).
  - All style-MLP params arrive in a single packed DMA; the MLP runs in fp32
    on the tensor engine; each layer's bias + leaky-relu is one scalar-engine
    Prelu activation.
  - Modulation is folded into the conv weights: w_mod = conv_w * s[b, ci]
    via per-partition-scaled fp32->bf16 activations, split by co-half so the
    first conv chain starts as soon as its half of the weights has landed.
  - The conv is 18 accumulating 128x128x512 bf16 matmuls per PSUM tile,
    one tile per 8 output rows, using 2D (row, col) rhs access patterns so
    only valid output columns are computed.
  - PSUM drains to a compact fp32 output tile; output DMAs are chunked per
    8-row group so the store pipeline finishes right behind the last matmul.
Host-side work is layout-only (transpose / reshape / pad / dtype cast).
"""

import numpy as np
import ml_dtypes

B, D_CAT, C_IN, C_OUT, K, H, W = 16, 512, 256, 256, 3, 64, 64
NCORES = 8
BL = B // NCORES          # samples per core (2)
CIT = C_IN // 128         # ci tiles (2)
COT = C_OUT // 128        # co tiles (2)
GW = W + 2                # padded grid width (66)
GH = H + 2                # padded grid height (66)
RG = 8                    # output rows per PSUM tile (8*64 = 512 columns)
NT = H // RG              # row-group tiles per (sample, co-tile) (8)
KKT = K * K * CIT         # matmuls per accumulation chain (18)
WHALF = K * K * 128       # weight columns per (ci_t, co_t) half (1152)

# packed MLP-param column offsets (per partition, fp32).
# pp1 = layer-1 deps (y, w0, b0); pp2 = later layers (w1, w2, b1, b2)
_PY = 0                       # y^T:   4 k-tiles x BL
_PW0 = _PY + 4 * BL           # w0^T:  4 k-tiles x 256
_PB0 = _PW0 + 4 * C_IN        # b0: CIT
_P1TOT = _PB0 + CIT
_PW1 = 0                      # w1^T:  2 k-tiles x 256
_PB1 = _PW1 + 2 * C_IN        # b1: CIT
_P2TOT = _PB1 + CIT
_PW2 = 0                      # w2^T:  2 k-tiles x 256
_PB2 = _PW2 + 2 * C_IN        # b2: CIT
_P3TOT = _PB2 + CIT

_BF16 = ml_dtypes.bfloat16
_COMPILED = None


def _build():
    import concourse.mybir as mybir
    import concourse.tile as tile
    from concourse import bacc

    bf16 = mybir.dt.bfloat16
    f32 = mybir.dt.float32
    Prelu = mybir.ActivationFunctionType.Prelu

    nc = bacc.Bacc("TRN2", target_bir_lowering=False, debug=False, num_devices=NCORES)

    pp1_in = nc.declare_dram_parameter("pp1", [128, _P1TOT], f32, isOutput=False)
    pp2_in = nc.declare_dram_parameter("pp2", [128, _P2TOT], f32, isOutput=False)
    pp3_in = nc.declare_dram_parameter("pp3", [128, _P3TOT], f32, isOutput=False)
    wf_in = nc.declare_dram_parameter("wf", [CIT, COT, 128, WHALF], bf16, isOutput=False)
    xb_in = nc.declare_dram_parameter("xb", [BL, CIT, 128, GH * GW], bf16, isOutput=False)
    out_ext = nc.declare_dram_parameter("out", [BL, COT, 128, H * W], f32, isOutput=True)

    with tile.TileContext(nc) as tc:
        with (
            tc.tile_pool(name="const", bufs=1) as cpool,
            tc.tile_pool(name="xpad", bufs=4) as padpool,
            tc.tile_pool(name="wmod", bufs=8) as wmpool,
            tc.tile_pool(name="osb", bufs=3) as opool,
            tc.tile_pool(name="cpsum", bufs=8, space="PSUM") as cpsum,
        ):
            # warm the scalar-engine activation table before the params land
            # so the first real Prelu doesn't pay the LoadActFuncSet latency
            warm = cpool.tile([128, 1], f32)
            nc.vector.memset(warm[:], 0.0)
            nc.scalar.activation(warm[:], warm[:], Prelu, bias=warm[:], scale=1.0, alpha=0.01)

            # ---- DMAs, ordered by first use; x and conv weights go through the
            # gpsimd SWDGE queue so they don't serialize behind the param DMAs
            # on the HWDGE path ----
            pp1_sb = cpool.tile([128, _P1TOT], f32)
            nc.sync.dma_start(pp1_sb[:], pp1_in[:])
            pp2_sb = cpool.tile([128, _P2TOT], f32)
            nc.sync.dma_start(pp2_sb[:], pp2_in[:])
            pp3_sb = cpool.tile([128, _P3TOT], f32)
            nc.sync.dma_start(pp3_sb[:], pp3_in[:])

            wf_sbs = {}
            for co_t in range(COT):          # co0 halves first: first chains use co_t=0
                for ci_t in range(CIT):
                    t = cpool.tile([128, WHALF], bf16, tag=f"wf{ci_t}{co_t}")
                    nc.gpsimd.dma_start(t[:], wf_in[ci_t, co_t])
                    wf_sbs[(ci_t, co_t)] = t
                if co_t == 0:
                    grids = {}
                    for b in range(BL):
                        for ci_t in range(CIT):
                            t = padpool.tile([128, GH * GW], bf16)
                            nc.gpsimd.dma_start(t[:], xb_in[b, ci_t])
                            grids[(b, ci_t)] = t[:].rearrange("p (a b) -> p a b", b=GW)

            # ---- style MLP (fp32): s^T per ci-tile in SBUF ----
            def mlp_layer(rhs_of_kt, kts, w_sb, w_base, bias_ap, out_sb):
                for ct in range(CIT):
                    mps = cpsum.tile([128, RG * W], f32, tag="cps")
                    for kt in range(kts):
                        nc.tensor.matmul(
                            mps[:, :BL],
                            w_sb[:, w_base + kt * C_IN + ct * 128 :][:, :128],
                            rhs_of_kt(kt),
                            start=(kt == 0),
                            stop=(kt == kts - 1),
                        )
                    nc.scalar.activation(
                        out_sb[:, ct * BL : (ct + 1) * BL],
                        mps[:, :BL],
                        Prelu,
                        bias=bias_ap(ct),
                        scale=1.0,
                        alpha=0.01,
                    )

            s0_sb = cpool.tile([128, CIT * BL], f32)
            s1_sb = cpool.tile([128, CIT * BL], f32)
            s_sb = cpool.tile([128, CIT * BL], f32)
            mlp_layer(
                lambda kt: pp1_sb[:, _PY + kt * BL : _PY + (kt + 1) * BL],
                4, pp1_sb, _PW0,
                lambda ct: pp1_sb[:, _PB0 + ct : _PB0 + ct + 1],
                s0_sb,
            )
            mlp_layer(
                lambda kt: s0_sb[:, kt * BL : (kt + 1) * BL],
                2, pp2_sb, _PW1,
                lambda ct: pp2_sb[:, _PB1 + ct : _PB1 + ct + 1],
                s1_sb,
            )
            mlp_layer(
                lambda kt: s1_sb[:, kt * BL : (kt + 1) * BL],
                2, pp3_sb, _PW2,
                lambda ct: pp3_sb[:, _PB2 + ct : _PB2 + ct + 1],
                s_sb,
            )

            # ---- modulated weights: w_mod[b, ci_t, co_t] = wf * s[b, ci] (bf16) ----
            w_mods = {}
            for b in range(BL):
                for co_t in range(COT):
                    for ci_t in range(CIT):
                        t = wmpool.tile([128, WHALF], bf16)
                        nc.vector.tensor_scalar_mul(
                            t[:],
                            wf_sbs[(ci_t, co_t)][:],
                            s_sb[:, ci_t * BL + b : ci_t * BL + b + 1],
                        )
                        w_mods[(b, ci_t, co_t)] = t

            # ---- conv: per (sample, co-tile): weight-outer over all 8 PSUM banks,
            # so each stationary weight tile is loaded once per 8 matmuls ----
            def conv_group(b, co_t, o_sb, ns):
                # weight-outer over the row-groups in `ns`: each stationary
                # weight tile is loaded once per len(ns) matmuls
                pss = [
                    cpsum.tile([128, RG * W], f32, name=f"cps_{b}_{co_t}_{n}", tag="cps")
                    for n in ns
                ]
                q = 0
                for ci_t in range(CIT):
                    g = grids[(b, ci_t)]
                    wm = w_mods[(b, ci_t, co_t)]
                    for ki in range(K):
                        for kj in range(K):
                            wt = wm[:, (ki * K + kj) * 128 : (ki * K + kj + 1) * 128]
                            for j, n in enumerate(ns):
                                nc.tensor.matmul(
                                    pss[j][:],
                                    wt,
                                    g[:, RG * n + ki : RG * n + ki + RG, kj : kj + W],
                                    start=(q == 0),
                                    stop=(q == KKT - 1),
                                )
                            q += 1
                for j, n in enumerate(ns):
                    copy = nc.vector.tensor_copy if n % 2 == 0 else nc.scalar.copy
                    copy(o_sb[:, RG * W * n : RG * W * (n + 1)], pss[j][:])
                    nc.sync.dma_start(
                        out_ext[b, co_t][:, RG * W * n : RG * W * (n + 1)],
                        o_sb[:, RG * W * n : RG * W * (n + 1)],
                    )

            for b in range(BL):
                for co_t in range(COT):
                    o_sb = opool.tile([128, H * W], f32, tag="osb")
                    last = b == BL - 1 and co_t == COT - 1
                    if last:
                        # split the final group so its drain/store burst
                        # overlaps the trailing chain's matmuls
                        conv_group(b, co_t, o_sb, list(range(NT - 1)))
                        conv_group(b, co_t, o_sb, [NT - 1])
                    else:
                        conv_group(b, co_t, o_sb, list(range(NT)))

    nc.compile()
    return nc


def _get_nc():
    global _COMPILED
    if _COMPILED is None:
        _COMPILED = _build()
    return _COMPILED


def _prep_in_maps(x, y, w0, b0, w1, b1, w2, b2, conv_w):
    x = np.ascontiguousarray(x, dtype=np.float32)
    y = np.ascontiguousarray(y, dtype=np.float32)

    # packed per-core-invariant params: pp1 = (y, w0, b0), pp2 = (w1, w2, b1, b2)
    pp1_shared = np.empty((128, _P1TOT), dtype=np.float32)
    pp1_shared[:, _PW0 : _PW0 + 4 * C_IN] = (
        w0.astype(np.float32).T.reshape(4, 128, C_IN).transpose(1, 0, 2).reshape(128, 4 * C_IN)
    )
    pp1_shared[:, _PB0 : _PB0 + CIT] = b0.astype(np.float32).reshape(CIT, 128).T
    pp2 = np.empty((128, _P2TOT), dtype=np.float32)
    pp2[:, _PW1 : _PW1 + 2 * C_IN] = (
        w1.astype(np.float32).T.reshape(2, 128, C_IN).transpose(1, 0, 2).reshape(128, 2 * C_IN)
    )
    pp2[:, _PB1 : _PB1 + CIT] = b1.astype(np.float32).reshape(CIT, 128).T
    pp3 = np.empty((128, _P3TOT), dtype=np.float32)
    pp3[:, _PW2 : _PW2 + 2 * C_IN] = (
        w2.astype(np.float32).T.reshape(2, 128, C_IN).transpose(1, 0, 2).reshape(128, 2 * C_IN)
    )
    pp3[:, _PB2 : _PB2 + CIT] = b2.astype(np.float32).reshape(CIT, 128).T

    # conv weights: (co_t, co, ci_t, ci, ki, kj) -> (ci_t, co_t, ci, (ki kj) co)
    wf = np.ascontiguousarray(
        conv_w.astype(np.float32)
        .reshape(COT, 128, CIT, 128, K, K)
        .transpose(2, 0, 3, 4, 5, 1)
        .reshape(CIT, COT, 128, WHALF)
    ).astype(_BF16)

    xb_all = np.zeros((B, CIT, 128, GH, GW), dtype=_BF16)
    xb_all[:, :, :, 1 : H + 1, 1 : W + 1] = x.reshape(B, CIT, 128, H, W)
    xb_all = xb_all.reshape(B, CIT, 128, GH * GW)

    in_maps = []
    for c in range(NCORES):
        sl = slice(c * BL, (c + 1) * BL)
        pp1 = pp1_shared.copy()
        pp1[:, _PY : _PY + 4 * BL] = (
            y[sl].T.reshape(4, 128, BL).transpose(1, 0, 2).reshape(128, 4 * BL)
        )
        in_maps.append(
            {
                "pp1": pp1,
                "pp2": pp2,
                "pp3": pp3,
                "wf": wf,
                "xb": np.ascontiguousarray(xb_all[sl]),
            }
        )
    return in_maps


def _run(in_maps, trace=False):
    from concourse.bass_utils import run_bass_kernel_spmd

    nc = _get_nc()
    res = run_bass_kernel_spmd(nc, in_maps, list(range(NCORES)), trace=trace)
    out = np.concatenate(
        [res.results[c]["out"].reshape(BL, C_OUT, H, W) for c in range(NCORES)], axis=0
    ).astype(np.float32, copy=False)
    return out, res


def kernel(x, y, w0, b0, w1, b1, w2, b2, conv_w):
    in_maps = _prep_in_maps(x, y, w0, b0, w1, b1, w2, b2, conv_w)
    out, _ = _run(in_maps, trace=False)
    return out
